# revision 3
# baseline (speedup 1.0000x reference)
"""Distributed multi-head attention kernel for 8 Trainium2 NeuronCores.

Problem: x[2,2048,768] @ Wqkv[768,2304] + bqkv -> 12-head attention -> @ Wproj + bproj.

Sharding: data-parallel over batch (2) x sequence-quarter (4) = 8 cores.
Each core computes K/V for its whole batch (redundant across the 4 cores
sharing a batch -- avoids collectives), attention + projection for its own
512 query rows.

All matmuls run in bf16 (fp32 PSUM accumulation). Softmax runs without
max-subtraction (scores are O(1) for this data regime), with the denominator
computed by a ones-column folded into the V matmul and divided out after
the attention-value matmul (projection is linear, normalization per-head
happens right before projection).
"""

import numpy as np
import ml_dtypes

B = 2
L = 2048
D = 768
H = 12
HD = 64
SCALE = HD ** -0.5
N_CORES = 8
LQ = L // 4  # 512 query rows per core

_CACHED = {}


def _build_nc():
    from contextlib import ExitStack

    import concourse.bass as bass
    import concourse.mybir as mybir
    import concourse.tile as tile
    from concourse import bacc

    F32 = mybir.dt.float32
    BF16 = mybir.dt.bfloat16
    Alu = mybir.AluOpType
    Act = mybir.ActivationFunctionType

    nc = bacc.Bacc(target_bir_lowering=False)

    xT_h = nc.declare_dram_parameter("xT", [D, L], BF16, isOutput=False)
    xTq_h = nc.declare_dram_parameter("xTq", [D, LQ], BF16, isOutput=False)
    wqkv_h = nc.declare_dram_parameter("wqkv", [D, 3 * D], BF16, isOutput=False)
    bqkv_h = nc.declare_dram_parameter("bqkv", [3 * D], F32, isOutput=False)
    wp_h = nc.declare_dram_parameter("wproj2", [HD, H, D], BF16, isOutput=False)
    bp_h = nc.declare_dram_parameter("bproj", [D], F32, isOutput=False)
    y_h = nc.declare_dram_parameter("y", [LQ, D], F32, isOutput=True)

    DT = D // 128      # 6 tiles of the contraction dim
    LT = L // 128      # 16 key tiles
    JG = 2             # j-tiles per exp group (psum banks per S tile)

    with tile.TileContext(nc) as tc:
        with tc.tile_pool(name="persist", bufs=1) as pp:
            xT_sb = pp.tile([128, DT, L], BF16)
            xTq_sb = pp.tile([128, DT, LQ], BF16)
            wqkv_sb = pp.tile([128, DT, 3 * D], BF16)
            wp_sb = pp.tile([HD, H, D], BF16)
            bias_sb = pp.tile([128, 18], F32)
            bv_sb = pp.tile([128, D], F32)
            bp_sb = pp.tile([128, D], F32)
            KT_sb = pp.tile([128, DT, L], BF16)
            QT_sb = pp.tile([128, DT, LQ], BF16)
            V_sb = pp.tile([128, LT, H, HD + 1], BF16)
            OT_sb = pp.tile([HD, H, LQ], BF16)
            ones_sb = pp.tile([128, HD], F32)

            # ---- input DMAs ----
            nc.sync.dma_start(out=xT_sb, in_=xT_h[:].rearrange("(n p) l -> p n l", p=128))
            nc.sync.dma_start(out=xTq_sb, in_=xTq_h[:].rearrange("(n p) l -> p n l", p=128))
            nc.sync.dma_start(out=wqkv_sb, in_=wqkv_h[:].rearrange("(n p) c -> p n c", p=128))
            nc.sync.dma_start(out=wp_sb, in_=wp_h[:])
            nc.sync.dma_start(out=bias_sb, in_=bqkv_h[:].rearrange("(n p) -> p n", p=128))
            bv_src = bqkv_h[2 * D:3 * D]
            nc.gpsimd.dma_start(
                out=bv_sb,
                in_=bass.AP(tensor=bv_src.tensor, offset=bv_src.offset,
                            ap=[[0, 128]] + list(bv_src.ap)),
            )
            bp_src = bp_h[:]
            nc.gpsimd.dma_start(
                out=bp_sb,
                in_=bass.AP(tensor=bp_src.tensor, offset=bp_src.offset,
                            ap=[[0, 128]] + list(bp_src.ap)),
            )
            nc.vector.memset(ones_sb, 1.0)
            nc.vector.memset(V_sb[:, :, :, HD], 1.0)

            # ---- phase 1+2: K^T [c,l], Q^T [c,l], V [l,c] ----
            with tc.tile_pool(name="ps1", bufs=4, space="PSUM") as ps1:
                # K^T: 6 c-tiles x 4 l-chunks of 512
                for kt in range(DT):
                    for lc in range(4):
                        ps = ps1.tile([128, 512], F32)
                        for dt in range(DT):
                            nc.tensor.matmul(
                                ps,
                                wqkv_sb[:, dt, D + kt * 128:D + (kt + 1) * 128],
                                xT_sb[:, dt, lc * 512:(lc + 1) * 512],
                                start=(dt == 0), stop=(dt == DT - 1),
                            )
                        nc.vector.tensor_scalar_add(
                            KT_sb[:, kt, lc * 512:(lc + 1) * 512], ps,
                            bias_sb[:, 6 + kt:7 + kt])
                # Q^T: 6 c-tiles x 1 l-chunk of 512
                for kt in range(DT):
                    ps = ps1.tile([128, 512], F32)
                    for dt in range(DT):
                        nc.tensor.matmul(
                            ps,
                            wqkv_sb[:, dt, kt * 128:(kt + 1) * 128],
                            xTq_sb[:, dt, :],
                            start=(dt == 0), stop=(dt == DT - 1),
                        )
                    nc.vector.tensor_scalar_add(
                        QT_sb[:, kt, :], ps, bias_sb[:, kt:kt + 1])
                # V: 16 l-tiles x 2 halves of 384 cols
                for lt in range(LT):
                    for hf in range(2):
                        ps = ps1.tile([128, 512], F32)
                        for dt in range(DT):
                            nc.tensor.matmul(
                                ps[:, :384],
                                xT_sb[:, dt, lt * 128:(lt + 1) * 128],
                                wqkv_sb[:, dt, 2 * D + hf * 384:2 * D + (hf + 1) * 384],
                                start=(dt == 0), stop=(dt == DT - 1),
                            )
                        nc.vector.tensor_tensor(
                            V_sb[:, lt, hf * 6:(hf + 1) * 6, 0:HD],
                            ps[:, :384].rearrange("p (h d) -> p h d", h=6),
                            bv_sb[:, hf * 384:(hf + 1) * 384].rearrange(
                                "p (h d) -> p h d", h=6),
                            Alu.add,
                        )

            # ---- phase 3: per-head attention ----
            with (
                tc.tile_pool(name="ps_s", bufs=2, space="PSUM") as ps_s,
                tc.tile_pool(name="ps_o", bufs=2, space="PSUM") as ps_o,
                tc.tile_pool(name="ps_rb", bufs=2, space="PSUM") as ps_rb,
                tc.tile_pool(name="ptp", bufs=2) as ptp,
                tc.tile_pool(name="rtp", bufs=2) as rtp,
            ):
                for h in range(H):
                    p0 = (h % 2) * 64
                    ct = h // 2
                    PT = ptp.tile([128, LT, LQ], BF16)
                    # S^T[j, i] = sum_d K^T[d, j] Q^T[d, i]   (contract 64)
                    for g in range(LT // JG):
                        sps = ps_s.tile([128, JG, LQ], F32)
                        for t in range(JG):
                            jt = JG * g + t
                            nc.tensor.matmul(
                                sps[:, t, :],
                                KT_sb[p0:p0 + 64, ct, jt * 128:(jt + 1) * 128],
                                QT_sb[p0:p0 + 64, ct, :],
                                start=True, stop=True,
                            )
                        nc.scalar.activation(
                            PT[:, JG * g:JG * (g + 1), :], sps, Act.Exp,
                            scale=SCALE)
                    # out^T[(d|1), i] = sum_j [V|1][j, d] P^T[j, i]
                    ops = ps_o.tile([128, LQ], F32)
                    for jt in range(LT):
                        nc.tensor.matmul(
                            ops[:HD + 1, :],
                            V_sb[:, jt, h, :],
                            PT[:, jt, :],
                            start=(jt == 0), stop=(jt == LT - 1),
                        )
                    # normalize: r = 1/denom broadcast via k=1 matmul
                    rt = rtp.tile([128, LQ], F32)
                    rts = rtp.tile([128, LQ], F32)
                    nc.vector.tensor_copy(rts[HD:HD + 1, :], ops[HD:HD + 1, :])
                    nc.vector.reciprocal(
                        out=rt[HD:HD + 1, :], in_=rts[HD:HD + 1, :])
                    rbp = ps_rb.tile([128, LQ], F32)
                    nc.tensor.matmul(
                        rbp[:HD, :], ones_sb[HD:HD + 1, :], rt[HD:HD + 1, :],
                        start=True, stop=True)
                    rbs = rtp.tile([HD, LQ], F32)
                    nc.vector.tensor_copy(rbs, rbp[:HD, :])
                    nc.vector.tensor_tensor(
                        OT_sb[:, h, :], ops[:HD, :], rbs, Alu.mult)

            # ---- phase 4: projection ----
            with (
                tc.tile_pool(name="ps4", bufs=4, space="PSUM") as ps4,
                tc.tile_pool(name="yp", bufs=2) as yp,
            ):
                y_r = y_h[:].rearrange("(n p) e -> p n e", p=128)
                for ic in range(LQ // 128):
                    for eh in range(2):
                        ps = ps4.tile([128, 512], F32)
                        for h in range(H):
                            nc.tensor.matmul(
                                ps[:, :384],
                                OT_sb[:, h, ic * 128:(ic + 1) * 128],
                                wp_sb[:, h, eh * 384:(eh + 1) * 384],
                                start=(h == 0), stop=(h == H - 1),
                            )
                        yt = yp.tile([128, 384], F32)
                        nc.vector.tensor_tensor(
                            yt, ps[:, :384], bp_sb[:, eh * 384:(eh + 1) * 384],
                            Alu.add)
                        nc.sync.dma_start(
                            out=y_r[:, ic, eh * 384:(eh + 1) * 384], in_=yt)

    nc.finalize()
    return nc


def _get_nc():
    if "nc" not in _CACHED:
        _CACHED["nc"] = _build_nc()
    return _CACHED["nc"]


def _make_in_maps(x, Wqkv, bqkv, Wproj, bproj):
    bf16 = ml_dtypes.bfloat16
    x = np.asarray(x, dtype=np.float32)
    wqkv16 = np.ascontiguousarray(np.asarray(Wqkv, dtype=np.float32).astype(bf16))
    bqkv32 = np.ascontiguousarray(np.asarray(bqkv, dtype=np.float32))
    wp2 = np.ascontiguousarray(
        np.asarray(Wproj, dtype=np.float32).astype(bf16)
        .reshape(H, HD, D).transpose(1, 0, 2))
    bp32 = np.ascontiguousarray(np.asarray(bproj, dtype=np.float32))

    xT = [np.ascontiguousarray(x[b].T.astype(bf16)) for b in range(B)]
    in_maps = []
    for c in range(N_CORES):
        b, s = c // 4, c % 4
        in_maps.append({
            "xT": xT[b],
            "xTq": np.ascontiguousarray(xT[b][:, s * LQ:(s + 1) * LQ]),
            "wqkv": wqkv16,
            "bqkv": bqkv32,
            "wproj2": wp2,
            "bproj": bp32,
        })
    return in_maps


def run(inputs, trace=False):
    """Run the SPMD kernel. Returns (full_output [2,2048,768] f32, BassKernelResults)."""
    from concourse.bass_utils import run_bass_kernel_spmd

    nc = _get_nc()
    in_maps = _make_in_maps(**inputs)
    res = run_bass_kernel_spmd(nc, in_maps, list(range(N_CORES)), trace=trace)
    out = np.empty((B, L, D), dtype=np.float32)
    for c in range(N_CORES):
        b, s = c // 4, c % 4
        out[b, s * LQ:(s + 1) * LQ, :] = res.results[c]["y"]
    return out, res


def kernel(**inputs) -> np.ndarray:
    return run(inputs)[0]


# revision 5
# speedup vs baseline: 1.1909x; 1.1909x over previous
"""Distributed multi-head attention kernel for 8 Trainium2 NeuronCores.

Problem: x[2,2048,768] @ Wqkv[768,2304] + bqkv -> 12-head attention -> @ Wproj + bproj.

Sharding: data-parallel over batch (2) x sequence-quarter (4) = 8 cores.
Each core computes K/V for its whole batch (redundant across the 4 cores
sharing a batch -- avoids collectives), attention + projection for its own
512 query rows.

All matmuls are full-width 128-contract / 128-out (partial-array matmuls run
at half the PE clock): the per-head S^T matmul (contract = head_dim = 64) is
zero-padded via a K^T buffer whose complementary partition half is zeroed,
the attention-value matmul pads the stationary [V|ones] operand with junk
columns (junk output partitions are never read), and the projection pads its
contract dim with zeros.

Softmax runs without max-subtraction (scores are O(1) for this data regime).
The denominator is a ones-column folded into V; all 12 head denominators are
gathered into one PSUM tile via k=1 matmuls, inverted in a single DVE
reciprocal, broadcast back via k=12 matmuls, and multiplied into the context
right before projection (projection is linear, so normalize-late is exact).
"""

import numpy as np
import ml_dtypes

B = 2
L = 2048
D = 768
H = 12
HD = 64
SCALE = HD ** -0.5
N_CORES = 8
LQ = L // 4  # 512 query rows per core

_CACHED = {}


def _build_nc():
    import concourse.bass as bass
    import concourse.mybir as mybir
    import concourse.tile as tile
    from concourse import bacc

    F32 = mybir.dt.float32
    BF16 = mybir.dt.bfloat16
    Alu = mybir.AluOpType
    Act = mybir.ActivationFunctionType

    nc = bacc.Bacc(target_bir_lowering=False)

    xT_h = nc.declare_dram_parameter("xT", [D, L], BF16, isOutput=False)
    xTq_h = nc.declare_dram_parameter("xTq", [D, LQ], BF16, isOutput=False)
    wqkv_h = nc.declare_dram_parameter("wqkv", [D, 3 * D], BF16, isOutput=False)
    bqkv_h = nc.declare_dram_parameter("bqkv", [3 * D], F32, isOutput=False)
    wp_h = nc.declare_dram_parameter("wproj2", [HD, H, D], BF16, isOutput=False)
    sel_h = nc.declare_dram_parameter("selmat", [H, H * HD], F32, isOutput=False)
    bp_h = nc.declare_dram_parameter("bproj", [D], F32, isOutput=False)
    y_h = nc.declare_dram_parameter("y", [LQ, D], F32, isOutput=True)

    DT = D // 128      # 6 tiles of the qkv contraction dim
    LT = L // 128      # 16 key tiles
    JG = 2             # j-tiles per exp group (psum banks per S tile)
    VW = 65            # V block width per head (64 ctx + 1 ones)
    VPAD = 11 * VW + 128  # 843 -> pad V free dim so lhsT can read 128 cols

    with tile.TileContext(nc) as tc:
        with tc.tile_pool(name="persist", bufs=1) as pp:
            # persistent across the whole kernel
            KTz_sb = pp.tile([128, H, L], BF16)         # K^T per head, other parity half zeroed
            QT_sb = pp.tile([128, DT, LQ], BF16)
            V_sb = pp.tile([128, LT, VPAD + 5], BF16)   # [V_h | ones] blocks at h*65
            OT2_sb = pp.tile([128, H, LQ], BF16)        # ctx^T per head on rows 0:64, zeros 64:128
            wp_sb = pp.tile([128, H, D], BF16)          # Wproj rows per head; rows 64:128 zeroed
            bias_sb = pp.tile([128, 18], F32)
            bv_sb = pp.tile([128, D], F32)
            bp_sb = pp.tile([128, D], F32)
            eye_sb = pp.tile([128, H * H], F32)         # gather one-hots on partition 64
            sel_sb = pp.tile([H, H * HD], F32)          # bcast selectors, k=12
            Dsb = pp.tile([H, LQ], F32)
            Rsb = pp.tile([H, LQ], F32)

            # constants
            nc.vector.memset(KTz_sb, 0.0)
            nc.vector.memset(V_sb, 0.0)
            nc.vector.memset(OT2_sb[64:128, :, :], 0.0)
            nc.vector.memset(wp_sb[64:128, :, :], 0.0)
            nc.vector.memset(eye_sb, 0.0)
            nc.sync.dma_start(out=sel_sb, in_=sel_h[:])
            for h in range(H):
                nc.vector.memset(V_sb[:, :, h * VW + HD:h * VW + HD + 1], 1.0)
                nc.vector.memset(eye_sb[64:65, h * H + h:h * H + h + 1], 1.0)

            nc.sync.dma_start(out=wp_sb[0:HD, :, :], in_=wp_h[:])
            nc.sync.dma_start(out=bias_sb, in_=bqkv_h[:].rearrange("(n p) -> p n", p=128))
            bv_src = bqkv_h[2 * D:3 * D]
            nc.gpsimd.dma_start(
                out=bv_sb,
                in_=bass.AP(tensor=bv_src.tensor, offset=bv_src.offset,
                            ap=[[0, 128]] + list(bv_src.ap)),
            )
            bp_src = bp_h[:]
            nc.gpsimd.dma_start(
                out=bp_sb,
                in_=bass.AP(tensor=bp_src.tensor, offset=bp_src.offset,
                            ap=[[0, 128]] + list(bp_src.ap)),
            )

            # ---- phase 1+2: K^T (zero-padded per head), Q^T, V ----
            with tc.tile_pool(name="loadp", bufs=1) as lp:
                xT_sb = lp.tile([128, DT, L], BF16)
                xTq_sb = lp.tile([128, DT, LQ], BF16)
                wqkv_sb = lp.tile([128, DT, 3 * D], BF16)

                wq_r = wqkv_h[:].rearrange("(n p) c -> p n c", p=128)
                nc.sync.dma_start(out=wqkv_sb[:, :, D:2 * D], in_=wq_r[:, :, D:2 * D])
                nc.sync.dma_start(out=xT_sb, in_=xT_h[:].rearrange("(n p) l -> p n l", p=128))
                nc.sync.dma_start(out=xTq_sb, in_=xTq_h[:].rearrange("(n p) l -> p n l", p=128))
                nc.sync.dma_start(out=wqkv_sb[:, :, 0:D], in_=wq_r[:, :, 0:D])
                nc.sync.dma_start(out=wqkv_sb[:, :, 2 * D:3 * D], in_=wq_r[:, :, 2 * D:3 * D])

                with tc.tile_pool(name="ps1", bufs=4, space="PSUM") as ps1:
                    # K^T: 6 c-tiles x 4 l-chunks; evac splits head parities
                    for kt in range(DT):
                        for lc in range(4):
                            ps = ps1.tile([128, 512], F32)
                            for dt in range(DT):
                                nc.tensor.matmul(
                                    ps,
                                    wqkv_sb[:, dt, D + kt * 128:D + (kt + 1) * 128],
                                    xT_sb[:, dt, lc * 512:(lc + 1) * 512],
                                    start=(dt == 0), stop=(dt == DT - 1),
                                )
                            ls = slice(lc * 512, (lc + 1) * 512)
                            nc.vector.tensor_scalar_add(
                                KTz_sb[0:64, 2 * kt, ls], ps[0:64, :],
                                bias_sb[0:64, 6 + kt:7 + kt])
                            nc.vector.tensor_scalar_add(
                                KTz_sb[64:128, 2 * kt + 1, ls], ps[64:128, :],
                                bias_sb[64:128, 6 + kt:7 + kt])
                    # Q^T: 6 c-tiles x 1 l-chunk
                    for kt in range(DT):
                        ps = ps1.tile([128, 512], F32)
                        for dt in range(DT):
                            nc.tensor.matmul(
                                ps,
                                wqkv_sb[:, dt, kt * 128:(kt + 1) * 128],
                                xTq_sb[:, dt, :],
                                start=(dt == 0), stop=(dt == DT - 1),
                            )
                        nc.vector.tensor_scalar_add(
                            QT_sb[:, kt, :], ps, bias_sb[:, kt:kt + 1])
                    # V: 16 l-tiles x 2 halves of 384 cols
                    for lt in range(LT):
                        for hf in range(2):
                            ps = ps1.tile([128, 512], F32)
                            for dt in range(DT):
                                nc.tensor.matmul(
                                    ps[:, :384],
                                    xT_sb[:, dt, lt * 128:(lt + 1) * 128],
                                    wqkv_sb[:, dt, 2 * D + hf * 384:2 * D + (hf + 1) * 384],
                                    start=(dt == 0), stop=(dt == DT - 1),
                                )
                            nc.vector.tensor_tensor(
                                V_sb[:, lt, 390 * hf:390 * hf + 390].rearrange(
                                    "p (h c) -> p h c", c=VW)[:, :, 0:HD],
                                ps[:, :384].rearrange("p (h d) -> p h d", h=6),
                                bv_sb[:, hf * 384:(hf + 1) * 384].rearrange(
                                    "p (h d) -> p h d", h=6),
                                Alu.add,
                            )

            # ---- phase 3: per-head attention ----
            with (
                tc.tile_pool(name="ps_s", bufs=2, space="PSUM") as ps_s,
                tc.tile_pool(name="ps_o", bufs=2, space="PSUM") as ps_o,
                tc.tile_pool(name="ps_d", bufs=1, space="PSUM") as ps_d,
                tc.tile_pool(name="ptp", bufs=2) as ptp,
                tc.tile_pool(name="dtp", bufs=2) as dtp,
            ):
                D_ps = ps_d.tile([H, LQ], F32)
                for h in range(H):
                    PT = ptp.tile([128, LT, LQ], BF16)
                    # S^T[j, i] = sum_d K^T[d, j] Q^T[d, i]; zero-padded to k=128
                    for g in range(LT // JG):
                        sps = ps_s.tile([128, JG, LQ], F32, tag="sps")
                        for t in range(JG):
                            jt = JG * g + t
                            nc.tensor.matmul(
                                sps[:, t, :],
                                KTz_sb[:, h, jt * 128:(jt + 1) * 128],
                                QT_sb[:, h // 2, :],
                                start=True, stop=True,
                            )
                        nc.scalar.activation(
                            PT[:, JG * g:JG * (g + 1), :], sps, Act.Exp,
                            scale=SCALE)
                    # out^T[(d|1|junk), i] = sum_j [V|1|junk][j, :] P^T[j, i]
                    ops = ps_o.tile([128, LQ], F32)
                    for jt in range(LT):
                        nc.tensor.matmul(
                            ops,
                            V_sb[:, jt, h * VW:h * VW + 128],
                            PT[:, jt, :],
                            start=(jt == 0), stop=(jt == LT - 1),
                        )
                    # stash raw ctx + gather denominator row into D_ps[h]
                    nc.vector.tensor_copy(OT2_sb[0:HD, h, :], ops[0:HD, :])
                    dst = dtp.tile([128, LQ], F32)
                    nc.vector.tensor_copy(dst[HD:HD + 1, :], ops[HD:HD + 1, :])
                    nc.tensor.matmul(
                        D_ps[:, :], eye_sb[64:65, h * H:(h + 1) * H],
                        dst[HD:HD + 1, :],
                        start=(h == 0), stop=(h == H - 1),
                        skip_group_check=True,
                    )

                # normalize all heads: one reciprocal, then per-head bcast+mult
                nc.vector.tensor_copy(Dsb, D_ps)
                nc.vector.reciprocal(out=Rsb, in_=Dsb)
                for h in range(H):
                    rb = ps_s.tile([HD, LQ], F32, tag="sps")
                    nc.tensor.matmul(
                        rb, sel_sb[:, h * HD:(h + 1) * HD], Rsb,
                        start=True, stop=True)
                    nc.vector.tensor_tensor(
                        OT2_sb[0:HD, h, :], OT2_sb[0:HD, h, :], rb, Alu.mult)

            # ---- phase 4: projection (contract zero-padded to 128) ----
            with (
                tc.tile_pool(name="ps4", bufs=4, space="PSUM") as ps4,
                tc.tile_pool(name="yp", bufs=2) as yp,
            ):
                y_r = y_h[:].rearrange("(n p) e -> p n e", p=128)
                for ic in range(LQ // 128):
                    for eh in range(2):
                        ps = ps4.tile([128, 512], F32)
                        for h in range(H):
                            nc.tensor.matmul(
                                ps[:, :384],
                                OT2_sb[:, h, ic * 128:(ic + 1) * 128],
                                wp_sb[:, h, eh * 384:(eh + 1) * 384],
                                start=(h == 0), stop=(h == H - 1),
                            )
                        yt = yp.tile([128, 384], F32)
                        nc.vector.tensor_tensor(
                            yt, ps[:, :384], bp_sb[:, eh * 384:(eh + 1) * 384],
                            Alu.add)
                        nc.sync.dma_start(
                            out=y_r[:, ic, eh * 384:(eh + 1) * 384], in_=yt)

    nc.finalize()
    return nc


def _get_nc():
    if "nc" not in _CACHED:
        _CACHED["nc"] = _build_nc()
    return _CACHED["nc"]


def _make_in_maps(x, Wqkv, bqkv, Wproj, bproj):
    bf16 = ml_dtypes.bfloat16
    x = np.asarray(x, dtype=np.float32)
    wqkv16 = np.ascontiguousarray(np.asarray(Wqkv, dtype=np.float32).astype(bf16))
    bqkv32 = np.ascontiguousarray(np.asarray(bqkv, dtype=np.float32))
    wp2 = np.ascontiguousarray(
        np.asarray(Wproj, dtype=np.float32).astype(bf16)
        .reshape(H, HD, D).transpose(1, 0, 2))
    bp32 = np.ascontiguousarray(np.asarray(bproj, dtype=np.float32))
    selmat = np.zeros((H, H * HD), np.float32)
    for h in range(H):
        selmat[h, h * HD:(h + 1) * HD] = 1.0

    xT = [np.ascontiguousarray(x[b].T.astype(bf16)) for b in range(B)]
    in_maps = []
    for c in range(N_CORES):
        b, s = c // 4, c % 4
        in_maps.append({
            "xT": xT[b],
            "xTq": np.ascontiguousarray(xT[b][:, s * LQ:(s + 1) * LQ]),
            "wqkv": wqkv16,
            "bqkv": bqkv32,
            "wproj2": wp2,
            "bproj": bp32,
            "selmat": selmat,
        })
    return in_maps


def run(inputs, trace=False):
    """Run the SPMD kernel. Returns (full_output [2,2048,768] f32, BassKernelResults)."""
    from concourse.bass_utils import run_bass_kernel_spmd

    nc = _get_nc()
    in_maps = _make_in_maps(**inputs)
    res = run_bass_kernel_spmd(nc, in_maps, list(range(N_CORES)), trace=trace)
    out = np.empty((B, L, D), dtype=np.float32)
    for c in range(N_CORES):
        b, s = c // 4, c % 4
        out[b, s * LQ:(s + 1) * LQ, :] = res.results[c]["y"]
    return out, res


def kernel(**inputs) -> np.ndarray:
    return run(inputs)[0]


# revision 9
# speedup vs baseline: 1.4324x; 1.2028x over previous
"""Distributed multi-head attention kernel for 8 Trainium2 NeuronCores.

Problem: x[2,2048,768] @ Wqkv[768,2304] + bqkv -> 12-head attention -> @ Wproj + bproj.

Sharding: data-parallel over batch (2) x sequence-quarter (4) = 8 cores.
Each core computes K/V for its whole batch (redundant across the 4 cores
sharing a batch -- avoids collectives), attention + projection for its own
512 query rows.

All matmuls are full-width 128-contract / 128-out (partial-array matmuls run
at half the PE clock): the per-head S^T matmul (contract = head_dim = 64) is
zero-padded via a K^T buffer whose complementary partition half is zeroed,
the attention-value matmul pads the stationary [V|ones] operand with junk
columns (junk output partitions are never read), and the projection pads its
contract dim with zeros.

Softmax runs without max-subtraction (scores are O(1) for this data regime).
The denominator is a ones-column folded into V; all 12 head denominators are
gathered into one PSUM tile via zero-padded matmuls, inverted in a single DVE
reciprocal, broadcast back via zero-padded matmuls, and multiplied into the
context right before projection (projection is linear, so normalize-late is
exact).

Engine balance: large constant memsets run on GpSimd (otherwise idle), K^T
evacuation is split between ScalarE (even-parity half, which also adds the
per-partition bias) and VectorE so the PE is not evacuation-paced, and the
first two heads' score matmuls are emitted before the V projection so the
ScalarE exp pipeline starts ~90us earlier.
"""

import numpy as np
import ml_dtypes

B = 2
L = 2048
D = 768
H = 12
HD = 64
SCALE = HD ** -0.5
N_CORES = 8
LQ = L // 4  # 512 query rows per core

_CACHED = {}


def _build_nc():
    import concourse.bass as bass
    import concourse.mybir as mybir
    import concourse.tile as tile
    from concourse import bacc

    F32 = mybir.dt.float32
    BF16 = mybir.dt.bfloat16
    Alu = mybir.AluOpType
    Act = mybir.ActivationFunctionType

    nc = bacc.Bacc(target_bir_lowering=False)

    xT_h = nc.declare_dram_parameter("xT", [D, L], BF16, isOutput=False)
    xTq_h = nc.declare_dram_parameter("xTq", [D, LQ], BF16, isOutput=False)
    wqkv_h = nc.declare_dram_parameter("wqkv", [D, 3 * D], BF16, isOutput=False)
    bqkv_h = nc.declare_dram_parameter("bqkv", [3 * D], F32, isOutput=False)
    wp_h = nc.declare_dram_parameter("wproj2", [HD, H, D], BF16, isOutput=False)
    sel_h = nc.declare_dram_parameter("selmat", [H, H * 128], F32, isOutput=False)
    bp_h = nc.declare_dram_parameter("bproj", [D], F32, isOutput=False)
    y_h = nc.declare_dram_parameter("y", [LQ, D], F32, isOutput=True)

    DT = D // 128      # 6 tiles of the qkv contraction dim
    LT = L // 128      # 16 key tiles
    JG = 2             # j-tiles per exp group (psum banks per S tile)
    VW = 65            # V block width per head (64 ctx + 1 ones)
    VPAD = 11 * VW + 128  # pad V free dim so lhsT can read 128 cols

    with tile.TileContext(nc) as tc:
        with tc.tile_pool(name="persist", bufs=1) as pp:
            # persistent across the whole kernel
            KTz_sb = pp.tile([128, H, L], BF16)         # K^T per head, other parity half zeroed
            QT_sb = pp.tile([128, DT, LQ], BF16)
            V_sb = pp.tile([128, LT, VPAD + 5], BF16)   # [V_h | ones] blocks at h*65
            OT2_sb = pp.tile([128, H, LQ], BF16)        # ctx^T per head on rows 0:64, zeros 64:128
            bias_sb = pp.tile([128, 18], F32)
            bv_sb = pp.tile([128, D], F32)
            eye_sb = pp.tile([128, H * H], F32)         # gather one-hots on partition 64, rest zero
            sel_sb = pp.tile([128, H * 128], F32)       # bcast selectors on rows 0:12, rest zero
            dst_sb = pp.tile([128, LQ], F32)            # denom staging, only row 64 live
            Rsb = pp.tile([128, LQ], F32)               # 1/denom on rows 0:12, rest zero
            Dsb = pp.tile([H, LQ], F32)

            # constants (big zero-fills on the otherwise idle GpSimd engine)
            nc.gpsimd.memset(KTz_sb, 0.0)
            nc.gpsimd.memset(OT2_sb[64:128, :, :], 0.0)
            nc.gpsimd.memset(dst_sb, 0.0)
            nc.gpsimd.memset(Rsb, 0.0)
            nc.vector.memset(eye_sb, 0.0)
            nc.vector.memset(sel_sb, 0.0)
            nc.sync.dma_start(out=sel_sb[0:H, :], in_=sel_h[:])
            for h in range(H):
                nc.vector.memset(V_sb[:, :, h * VW + HD:h * VW + HD + 1], 1.0)
                nc.vector.memset(eye_sb[64:65, h * H + h:h * H + h + 1], 1.0)

            nc.sync.dma_start(out=bias_sb, in_=bqkv_h[:].rearrange("(n p) -> p n", p=128))
            bv_src = bqkv_h[2 * D:3 * D]
            nc.gpsimd.dma_start(
                out=bv_sb,
                in_=bass.AP(tensor=bv_src.tensor, offset=bv_src.offset,
                            ap=[[0, 128]] + list(bv_src.ap)),
            )
            with (
                tc.tile_pool(name="loadp", bufs=1) as lp,
                tc.tile_pool(name="ps_s", bufs=2, space="PSUM") as ps_s,
                tc.tile_pool(name="ps_o", bufs=2, space="PSUM") as ps_o,
                tc.tile_pool(name="ps_d", bufs=1, space="PSUM") as ps_d,
                tc.tile_pool(name="ptp", bufs=2) as ptp,
            ):
                xT_sb = lp.tile([128, DT, L], BF16)
                xTq_sb = lp.tile([128, DT, LQ], BF16)
                wqkv_sb = lp.tile([128, DT, 3 * D], BF16)

                wq_r = wqkv_h[:].rearrange("(n p) c -> p n c", p=128)
                nc.sync.dma_start(out=wqkv_sb[:, :, D:2 * D], in_=wq_r[:, :, D:2 * D])
                nc.sync.dma_start(out=xT_sb, in_=xT_h[:].rearrange("(n p) l -> p n l", p=128))
                nc.sync.dma_start(out=xTq_sb, in_=xTq_h[:].rearrange("(n p) l -> p n l", p=128))
                nc.sync.dma_start(out=wqkv_sb[:, :, 0:D], in_=wq_r[:, :, 0:D])
                nc.sync.dma_start(out=wqkv_sb[:, :, 2 * D:3 * D], in_=wq_r[:, :, 2 * D:3 * D])

                D_ps = ps_d.tile([H, LQ], F32)

                def ktqt_block(kt):
                    # K^T c-tile: evac halves split across ScalarE / VectorE
                    for lc in range(4):
                        ps = ps_s.tile([128, JG, LQ], F32, tag="sps")
                        for dt in range(DT):
                            nc.tensor.matmul(
                                ps[:, 0, :],
                                wqkv_sb[:, dt, D + kt * 128:D + (kt + 1) * 128],
                                xT_sb[:, dt, lc * 512:(lc + 1) * 512],
                                start=(dt == 0), stop=(dt == DT - 1),
                            )
                        ls = slice(lc * 512, (lc + 1) * 512)
                        nc.scalar.activation(
                            KTz_sb[0:64, 2 * kt, ls], ps[0:64, 0, :],
                            Act.Identity, bias=bias_sb[0:64, 6 + kt:7 + kt])
                        nc.vector.tensor_scalar_add(
                            KTz_sb[64:128, 2 * kt + 1, ls], ps[64:128, 0, :],
                            bias_sb[64:128, 6 + kt:7 + kt])
                    # Q^T c-tile
                    ps = ps_s.tile([128, JG, LQ], F32, tag="sps")
                    for dt in range(DT):
                        nc.tensor.matmul(
                            ps[:, 0, :],
                            wqkv_sb[:, dt, kt * 128:(kt + 1) * 128],
                            xTq_sb[:, dt, :],
                            start=(dt == 0), stop=(dt == DT - 1),
                        )
                    nc.vector.tensor_scalar_add(
                        QT_sb[:, kt, :], ps[:, 0, :], bias_sb[:, kt:kt + 1])

                def v_block(lt):
                    for hf in range(2):
                        ps = ps_o.tile([128, LQ], F32, tag="ops")
                        for dt in range(DT):
                            nc.tensor.matmul(
                                ps[:, :384],
                                xT_sb[:, dt, lt * 128:(lt + 1) * 128],
                                wqkv_sb[:, dt, 2 * D + hf * 384:2 * D + (hf + 1) * 384],
                                start=(dt == 0), stop=(dt == DT - 1),
                            )
                        nc.vector.tensor_tensor(
                            V_sb[:, lt, 390 * hf:390 * hf + 390].rearrange(
                                "p (h c) -> p h c", c=VW)[:, :, 0:HD],
                            ps[:, :384].rearrange("p (h d) -> p h d", h=6),
                            bv_sb[:, hf * 384:(hf + 1) * 384].rearrange(
                                "p (h d) -> p h d", h=6),
                            Alu.add,
                        )

                def s_block(h):
                    # S^T[j, i] = sum_d K^T[d, j] Q^T[d, i]; zero-padded to k=128
                    PT = ptp.tile([128, LT, LQ], BF16, tag="PT")
                    for g in range(LT // JG):
                        sps = ps_s.tile([128, JG, LQ], F32, tag="sps")
                        for t in range(JG):
                            jt = JG * g + t
                            nc.tensor.matmul(
                                sps[:, t, :],
                                KTz_sb[:, h, jt * 128:(jt + 1) * 128],
                                QT_sb[:, h // 2, :],
                                start=True, stop=True,
                            )
                        nc.scalar.activation(
                            PT[:, JG * g:JG * (g + 1), :], sps, Act.Exp,
                            scale=SCALE)
                    return PT

                def out_block(h, PT):
                    # out^T[(d|1|junk), i] = sum_j [V|1|junk][j, :] P^T[j, i]
                    ops = ps_o.tile([128, LQ], F32, tag="ops")
                    for jt in range(LT):
                        nc.tensor.matmul(
                            ops,
                            V_sb[:, jt, h * VW:h * VW + 128],
                            PT[:, jt, :],
                            start=(jt == 0), stop=(jt == LT - 1),
                        )
                    # stash raw ctx + gather denominator row into D_ps[h]
                    nc.vector.tensor_copy(OT2_sb[0:HD, h, :], ops[0:HD, :])
                    nc.vector.tensor_copy(dst_sb[HD:HD + 1, :], ops[HD:HD + 1, :])
                    nc.tensor.matmul(
                        D_ps[:, :], eye_sb[:, h * H:(h + 1) * H], dst_sb,
                        start=(h == 0), stop=(h == H - 1),
                        skip_group_check=True,
                    )

                # ---- schedule: K^T/Q^T, first two heads' scores, V, rest ----
                for kt in range(DT):
                    ktqt_block(kt)
                PT0 = s_block(0)
                PT1 = s_block(1)
                for lt in range(LT):
                    v_block(lt)
                out_block(0, PT0)
                out_block(1, PT1)
                for h in range(2, H):
                    PT = s_block(h)
                    out_block(h, PT)

                # normalize all heads: one reciprocal, then per-head bcast+mult
                nc.vector.tensor_copy(Dsb, D_ps)
                nc.vector.reciprocal(out=Rsb[0:H, :], in_=Dsb)
                for h in range(H):
                    rb = ps_s.tile([128, LQ], F32, tag="sps")
                    nc.tensor.matmul(
                        rb, sel_sb[:, h * 128:(h + 1) * 128], Rsb,
                        start=True, stop=True)
                    nc.vector.tensor_tensor(
                        OT2_sb[0:HD, h, :], OT2_sb[0:HD, h, :], rb[0:HD, :],
                        Alu.mult)

            # ---- projection (contract zero-padded to 128) ----
            with (
                tc.tile_pool(name="ps4", bufs=4, space="PSUM") as ps4,
                tc.tile_pool(name="yp", bufs=2) as yp,
                tc.tile_pool(name="wpp", bufs=1) as wpp,
            ):
                wp_sb = wpp.tile([128, H, D], BF16)
                bp_sb = wpp.tile([128, D], F32)
                nc.gpsimd.memset(wp_sb[64:128, :, :], 0.0)
                nc.sync.dma_start(out=wp_sb[0:HD, :, :], in_=wp_h[:])
                bp_src = bp_h[:]
                nc.gpsimd.dma_start(
                    out=bp_sb,
                    in_=bass.AP(tensor=bp_src.tensor, offset=bp_src.offset,
                                ap=[[0, 128]] + list(bp_src.ap)),
                )
                y_r = y_h[:].rearrange("(n p) e -> p n e", p=128)
                for ic in range(LQ // 128):
                    for eh in range(2):
                        ps = ps4.tile([128, 512], F32)
                        for h in range(H):
                            nc.tensor.matmul(
                                ps[:, :384],
                                OT2_sb[:, h, ic * 128:(ic + 1) * 128],
                                wp_sb[:, h, eh * 384:(eh + 1) * 384],
                                start=(h == 0), stop=(h == H - 1),
                            )
                        yt = yp.tile([128, 384], F32)
                        nc.vector.tensor_tensor(
                            yt, ps[:, :384], bp_sb[:, eh * 384:(eh + 1) * 384],
                            Alu.add)
                        nc.sync.dma_start(
                            out=y_r[:, ic, eh * 384:(eh + 1) * 384], in_=yt)

    nc.finalize()
    return nc


def _get_nc():
    if "nc" not in _CACHED:
        _CACHED["nc"] = _build_nc()
    return _CACHED["nc"]


def _make_in_maps(x, Wqkv, bqkv, Wproj, bproj):
    bf16 = ml_dtypes.bfloat16
    x = np.asarray(x, dtype=np.float32)
    wqkv16 = np.ascontiguousarray(np.asarray(Wqkv, dtype=np.float32).astype(bf16))
    bqkv32 = np.ascontiguousarray(np.asarray(bqkv, dtype=np.float32))
    wp2 = np.ascontiguousarray(
        np.asarray(Wproj, dtype=np.float32).astype(bf16)
        .reshape(H, HD, D).transpose(1, 0, 2))
    bp32 = np.ascontiguousarray(np.asarray(bproj, dtype=np.float32))
    selmat = np.zeros((H, H * 128), np.float32)
    for h in range(H):
        selmat[h, h * 128:(h + 1) * 128] = 1.0

    xT = [np.ascontiguousarray(x[b].T.astype(bf16)) for b in range(B)]
    in_maps = []
    for c in range(N_CORES):
        b, s = c // 4, c % 4
        in_maps.append({
            "xT": xT[b],
            "xTq": np.ascontiguousarray(xT[b][:, s * LQ:(s + 1) * LQ]),
            "wqkv": wqkv16,
            "bqkv": bqkv32,
            "wproj2": wp2,
            "bproj": bp32,
            "selmat": selmat,
        })
    return in_maps


def run(inputs, trace=False):
    """Run the SPMD kernel. Returns (full_output [2,2048,768] f32, BassKernelResults)."""
    from concourse.bass_utils import run_bass_kernel_spmd

    nc = _get_nc()
    in_maps = _make_in_maps(**inputs)
    res = run_bass_kernel_spmd(nc, in_maps, list(range(N_CORES)), trace=trace)
    out = np.empty((B, L, D), dtype=np.float32)
    for c in range(N_CORES):
        b, s = c // 4, c % 4
        out[b, s * LQ:(s + 1) * LQ, :] = res.results[c]["y"]
    return out, res


def kernel(**inputs) -> np.ndarray:
    return run(inputs)[0]
